# revision 1
# baseline (speedup 1.0000x reference)
"""Trainium2 Bass kernel: grouped-pointwise FFN with channel shuffle.

Computes (per batch b, all ops pointwise in T):
    h   = W1_grouped @ (x * mask) + b1          # G=4 block-diagonal GEMM
    h   = channel_shuffle(h, G)
    h   = gelu(h)                               # exact erf gelu
    out = (W2_grouped @ h + b2) * mask

Sharding: data-parallel over batch B=16 across 8 cores (2 batches/core).
Weights are replicated; no collectives.

Layout on device (channel-partition):
  GEMM1: lhsT = w1 block [K=128(cin/G), M=128(out-ch block)],
         rhs  = x tile [128, 512(T chunk)], PSUM out [128, 512].
  gelu+bias fused on ScalarE reading PSUM [128, 1024] spans (2 banks).
  Channel shuffle is free: GEMM2's weight blocks are pre-gathered on the
  host so that GEMM2 group g2 contracts directly over GEMM1's (g, m=g2)
  output tiles.
  GEMM2: accumulate 4 K-blocks into PSUM [128, 512]; drain with a single
  fused DVE op: out = (psum + b2) * mask.

Matmuls stream float32r (fp32 at 1 cycle/row vs 4 for float32; measured
end-to-end rel err ~2e-4 vs fp32 reference). All tensors feeding matmuls
are typed float32r end-to-end (BIR verifier requirement).

DMA: inputs/weights on the SP HWDGE ring (small tensors first, weights
chunked in use-order), outputs on the otherwise-idle GpSimd SWDGE ring.
A burst of tiny warm-up matmuls keeps the PE HAM clock-gate warm before
the first real GEMM.
"""

import numpy as np

import concourse.mybir as mybir
import concourse.tile as tile
from concourse import bacc
from concourse import bass_utils

F32 = mybir.dt.float32
F32R = mybir.dt.float32r

N_CORES = 8
B, CIN, T = 16, 512, 2048
H, COUT, G = 2048, 512, 4
BPC = B // N_CORES        # batches per core
CH = 512                  # T chunk (= max fp32 matmul free dim = 1 PSUM bank)
NCH = T // CH             # 4 chunks
MB = (H // G) // 128      # 4 output-channel blocks per group in GEMM1
GELU_W = 1024             # ACT op width (2 PSUM banks)
XCH = 512                 # x / out DMA chunk width
N_WARMUP = 12             # tiny matmuls to warm the PE clock gate

MM_DT = F32R

_compiled = {}


def _build(mm_dt):
    nc = bacc.Bacc(
        "TRN2", target_bir_lowering=False, debug=False, num_devices=N_CORES
    )
    xs = nc.dram_tensor("xs", [BPC * G, 128, T], mm_dt, kind="ExternalInput").ap()
    mkr = nc.dram_tensor("mkr", [BPC, T], mm_dt, kind="ExternalInput").ap()
    ones = nc.dram_tensor("ones", [1, 128], mm_dt, kind="ExternalInput").ap()
    # w1t columns are (m, g, o)-major so the m=0 block is one contiguous
    # 512-col DMA needed first; w2t columns are (g2, g, o)-major.
    w1t = nc.dram_tensor("w1t", [128, G * MB * 128], mm_dt, kind="ExternalInput").ap()
    w2t = nc.dram_tensor("w2t", [128, G * G * 128], mm_dt, kind="ExternalInput").ap()
    b1t = nc.dram_tensor("b1t", [128, G * MB], F32, kind="ExternalInput").ap()
    b2t = nc.dram_tensor("b2t", [128, G], F32, kind="ExternalInput").ap()
    outs = nc.dram_tensor("outs", [BPC * G, 128, T], F32, kind="ExternalOutput").ap()

    with tile.TileContext(nc) as tc:
        with (
            tc.tile_pool(name="consts", bufs=1) as cpool,
            tc.tile_pool(name="xp", bufs=BPC * G) as xpool,
            tc.tile_pool(name="mbcp", bufs=2) as mbpool,
            tc.tile_pool(name="mkrp", bufs=2) as mkrpool,
            tc.tile_pool(name="hp", bufs=2 * G) as hpool,
            tc.tile_pool(name="op", bufs=2) as opool,
            tc.tile_pool(name="ps1p", bufs=3, space="PSUM") as ps1pool,
            tc.tile_pool(name="ps2p", bufs=2, space="PSUM") as ps2pool,
        ):
            # ones first (warm-up + mask broadcast depend on it)
            ones_sb = cpool.tile([1, 128], mm_dt)
            nc.sync.dma_start(ones_sb, ones)

            # PE warm-up: tiny matmuls on the ones row keep the HAM
            # activity window busy while real inputs stream in.
            wps = ps2pool.tile([128, 128], F32, tag="ps2", name="wps")
            for i in range(N_WARMUP):
                nc.tensor.matmul(
                    wps[:, 0:128], ones_sb, ones_sb, start=True, stop=True
                )

            w1_sb = cpool.tile([128, G * MB * 128], mm_dt)
            w2_sb = cpool.tile([128, G * G * 128], mm_dt)

            x_sb = [[None] * G for _ in range(BPC)]
            mask_bc = [None] * BPC

            def prep_batch(b):
                # mask row -> broadcast across 128 partitions via K=1
                # f32r matmuls, chunk by chunk; x loads and mask muls are
                # chunked so the first GEMM1 matmul can start early.
                mkrow = mkrpool.tile([1, T], mm_dt, tag="mkr", name="mkrow")
                nc.sync.dma_start(mkrow, mkr[b : b + 1, :])
                mbc = mbpool.tile([128, T], F32, tag="mbc", name="mbc")
                for c in range(NCH):
                    cs = slice(c * CH, (c + 1) * CH)
                    psb = ps2pool.tile([128, CH], F32, tag="ps2", name="psb")
                    nc.tensor.matmul(
                        psb, ones_sb, mkrow[:, cs], start=True, stop=True
                    )
                    nc.vector.tensor_copy(mbc[:, cs], psb)
                mask_bc[b] = mbc

            def load_x(b, g, split_first=False, ring=None):
                # ring: engine issuing the DMAs; later groups go out on
                # the otherwise-idle GpSimd SWDGE ring so both rings
                # issue in parallel during the head
                ring = nc.sync if ring is None else ring
                xt = xpool.tile([128, T], mm_dt, tag="x", name="xt")
                start_c = 0
                if split_first:
                    # first chunk as two 128KB DMAs so they land on
                    # parallel queues and GEMM1 can start sooner
                    for h in range(2):
                        hs = slice(h * (XCH // 2), (h + 1) * (XCH // 2))
                        ring.dma_start(xt[:, hs], xs[b * G + g][:, hs])
                        nc.vector.tensor_mul(
                            xt[:, hs], xt[:, hs], mask_bc[b][:, hs]
                        )
                    start_c = 1
                for c in range(start_c, T // XCH):
                    cs = slice(c * XCH, (c + 1) * XCH)
                    ring.dma_start(xt[:, cs], xs[b * G + g][:, cs])
                    nc.vector.tensor_mul(
                        xt[:, cs], xt[:, cs], mask_bc[b][:, cs]
                    )
                x_sb[b][g] = xt

            def load_w1(m):
                ws = slice(m * G * 128, (m + 1) * G * 128)
                nc.sync.dma_start(w1_sb[:, ws], w1t[:, ws])

            def load_w2(g2):
                ws = slice(g2 * G * 128, (g2 + 1) * G * 128)
                nc.sync.dma_start(w2_sb[:, ws], w2t[:, ws])

            def gemm1_g(b, m, g):
                # one h tile (g) for (b, m), gelu+bias fused on drain
                ht = hpool.tile([128, T], mm_dt, tag="h", name="ht")
                w_ap = w1_sb[:, (m * G + g) * 128 : (m * G + g + 1) * 128]
                for half in range(T // GELU_W):
                    ps1 = ps1pool.tile([128, GELU_W], F32, tag="ps1", name="ps1")
                    for cc in range(GELU_W // CH):
                        c = half * (GELU_W // CH) + cc
                        nc.tensor.matmul(
                            ps1[:, cc * CH : (cc + 1) * CH],
                            w_ap,
                            x_sb[b][g][:, c * CH : (c + 1) * CH],
                            start=True, stop=True,
                        )
                    nc.scalar.activation(
                        ht[:, half * GELU_W : (half + 1) * GELU_W],
                        ps1,
                        mybir.ActivationFunctionType.Gelu,
                        bias=b1_sb[:, m * G + g : m * G + g + 1],
                        scale=1.0,
                    )
                return ht

            def gemm2_chunk(b, g2, hts, ot, c, och=XCH * 2):
                cs = slice(c * CH, (c + 1) * CH)
                ps2 = ps2pool.tile([128, CH], F32, tag="ps2", name="ps2")
                for g in range(G):
                    nc.tensor.matmul(
                        ps2,
                        w2_sb[:, (g2 * G + g) * 128 : (g2 * G + g + 1) * 128],
                        hts[g][:, cs],
                        start=(g == 0), stop=(g == G - 1),
                    )
                # out = (psum + b2) * mask, single fused DVE op
                nc.vector.scalar_tensor_tensor(
                    ot[:, cs],
                    ps2,
                    b2_sb[:, g2 : g2 + 1],
                    mask_bc[b][:, cs],
                    op0=mybir.AluOpType.add,
                    op1=mybir.AluOpType.mult,
                )
                if ((c + 1) * CH) % och == 0:
                    os_ = slice((c + 1) * CH - och, (c + 1) * CH)
                    nc.sync.dma_start(outs[b * G + g2][:, os_], ot[:, os_])

            # head: mask prep + first x tiles, weight blocks in use-order;
            # batch 1 is prefetched entirely up front too (its broadcast
            # matmuls double as PE warm-up while DMAs stream)
            prep_batch(0)
            load_x(0, 0, split_first=True)
            load_x(0, 2, split_first=True, ring=nc.gpsimd)
            b1_sb = cpool.tile([128, G * MB], F32)
            nc.sync.dma_start(b1_sb, b1t)
            b2_sb = cpool.tile([128, G], F32)
            nc.sync.dma_start(b2_sb, b2t)
            load_w1(0)
            load_x(0, 1)
            load_x(0, 3, ring=nc.gpsimd)
            load_w1(1)
            load_w1(2)
            load_w1(3)
            for g2 in range(G):
                load_w2(g2)

            # software pipeline over (b, m): GEMM2 chunks of iteration i-1
            # are interleaved between GEMM1 groups of iteration i so PE
            # alternates with ScalarE instead of stalling on gelu.
            prev = None
            for b in range(BPC):
                for m in range(MB):
                    hts = []
                    if prev is not None:
                        pot = opool.tile([128, T], F32, tag="o", name="pot")
                    for g in range(G):
                        hts.append(gemm1_g(b, m, g))
                        if prev is not None:
                            gemm2_chunk(prev[0], prev[1], prev[2], pot, g)
                    prev = (b, m, hts)
                    if b + 1 < BPC and m == 1:
                        prep_batch(b + 1)
                        for g in range(G):
                            load_x(
                                b + 1, g,
                                ring=nc.gpsimd if g >= 2 else None,
                            )
            pot = opool.tile([128, T], F32, tag="o", name="pot")
            for c in range(NCH):
                gemm2_chunk(prev[0], prev[1], prev[2], pot, c, och=CH)

    nc.compile()
    return nc


def get_nc(mm_dt=None):
    mm_dt = MM_DT if mm_dt is None else mm_dt
    if mm_dt not in _compiled:
        _compiled[mm_dt] = _build(mm_dt)
    return _compiled[mm_dt]


def prep_inputs(x, x_mask, w1, b1, w2, b2):
    """Host-side layout prep. Returns per-core in_maps."""
    x = np.ascontiguousarray(np.asarray(x, dtype=np.float32))
    x_mask = np.asarray(x_mask, dtype=np.float32)
    w1 = np.asarray(w1, dtype=np.float32)
    b1 = np.asarray(b1, dtype=np.float32)
    w2 = np.asarray(w2, dtype=np.float32)
    b2 = np.asarray(b2, dtype=np.float32)

    # w1 [H, CIN/G] -> lhsT blocks [i, (m, g, o)]
    w1r = w1.reshape(G, MB, 128, CIN // G)          # g, m, o, i
    w1t = np.ascontiguousarray(
        np.transpose(w1r, (3, 1, 0, 2)).reshape(128, G * MB * 128)
    )
    # w2 [COUT, H/G] -> lhsT blocks [i_local, (g2, g, o)]
    # GEMM2 group g2 contracts h tile (g, m=g2) row r against
    # w2[g2*128+o, r*4+g] (channel shuffle pre-applied).
    w2r = w2.reshape(G, 128, 128, G)                # g2, o, r, g
    w2t = np.ascontiguousarray(
        np.transpose(w2r, (2, 0, 3, 1)).reshape(128, G * G * 128)
    )
    b1tt = np.ascontiguousarray(
        b1.reshape(G, MB, 128).transpose(2, 1, 0).reshape(128, G * MB)
    )
    b2tt = np.ascontiguousarray(b2.reshape(G, 128).T)
    ones = np.ones((1, 128), np.float32)

    xr = x.reshape(N_CORES, BPC * G, 128, T)
    mr = x_mask.reshape(N_CORES, BPC, T)

    in_maps = []
    for k in range(N_CORES):
        mk_k = np.ascontiguousarray(mr[k])
        in_maps.append(
            {
                "xs": np.ascontiguousarray(xr[k]),
                "mkr": mk_k,
                "ones": ones,
                "w1t": w1t,
                "w2t": w2t,
                "b1t": b1tt,
                "b2t": b2tt,
            }
        )
    return in_maps


def assemble_output(results):
    """results: list of 8 dicts with 'outs' [BPC*G, 128, T]."""
    parts = [r["outs"].reshape(BPC, G * 128, T) for r in results]
    return np.concatenate(parts, axis=0).astype(np.float32)


def kernel(x, x_mask, w1, b1, w2, b2, n_groups):
    assert int(n_groups) == G
    import os

    # NTFF tracing needs antenv.axon_hooks, absent on this image; make
    # sure an inherited BASS_TRACE can't push us onto that path.
    os.environ["BASS_NEVER_TRACE"] = "1"
    nc = get_nc()
    in_maps = prep_inputs(x, x_mask, w1, b1, w2, b2)
    res = bass_utils.run_bass_kernel_spmd(
        nc, in_maps, core_ids=list(range(N_CORES))
    )
    return assemble_output(res.results)



# revision 4
# speedup vs baseline: 1.1733x; 1.1733x over previous
"""Trainium2 Bass kernel: grouped-pointwise FFN with channel shuffle.

Computes (per batch b, all ops pointwise in T):
    h   = W1_grouped @ x + b1                   # G=4 block-diagonal GEMM
    h   = channel_shuffle(h, G)
    h   = gelu(h)                               # exact erf gelu
    out = (W2_grouped @ h + b2) * mask

The reference computes mask*(f(mask*x)); for binary masks (the only
semantically valid values for a sequence mask) this equals mask*f(x),
so the input-side mask multiply is dropped and masking is applied only
on the output.

Sharding: data-parallel over batch B=16 across 8 cores (2 batches/core).
Weights are replicated; no collectives.

Layout on device (channel-partition):
  GEMM1: lhsT = w1 block [K=128(cin/G), M=128(out-ch block)],
         rhs  = x tile [128, 512(T chunk)], PSUM out [128, 512].
  gelu+bias fused on ScalarE reading PSUM [128, 1024] spans (2 banks).
  Channel shuffle is free: GEMM2's weight blocks are pre-gathered on the
  host so that GEMM2 group g2 contracts directly over GEMM1's (g, m=g2)
  output tiles.
  GEMM2: accumulate 4 K-blocks into PSUM [128, 512]; drain with a single
  fused DVE op: out = (psum + b2) * mask.

All matmul operands are float16 (1 cycle/row on PE, half the DMA bytes
and half the LDWEIGHTS time of fp32); PSUM accumulation is fp32.
Measured end-to-end rel err ~5e-4 vs the fp32 reference. Outputs are
stored fp16 and upcast on the host.

DMA: weights/x for batch 0 issued first on both rings (SP HWDGE +
GpSimd SWDGE) so GEMM1 starts ~6us in; outputs drain on the SP ring
which is idle after the head. A burst of tiny warm-up matmuls keeps the
PE p-state ramp warm while the first inputs stream in.
"""

import numpy as np

import concourse.mybir as mybir
import concourse.tile as tile
from concourse import bacc
from concourse import bass_utils

F32 = mybir.dt.float32
F16 = mybir.dt.float16

N_CORES = 8
B, CIN, T = 16, 512, 2048
H, COUT, G = 2048, 512, 4
BPC = B // N_CORES        # batches per core
CH = 512                  # T chunk (= 1 PSUM bank of fp32)
NCH = T // CH             # 4 chunks
MB = (H // G) // 128      # 4 output-channel blocks per group in GEMM1
GELU_W = 1024             # ACT op width (2 PSUM banks)
N_WARMUP = 24             # tiny matmuls to warm the PE clock gate

MM_DT = F16

_compiled = {}


def _build(mm_dt):
    nc = bacc.Bacc(
        "TRN2", target_bir_lowering=False, debug=False, num_devices=N_CORES
    )
    xs = nc.dram_tensor("xs", [BPC * G, 128, T], mm_dt, kind="ExternalInput").ap()
    mkr = nc.dram_tensor("mkr", [BPC, T], mm_dt, kind="ExternalInput").ap()
    ones = nc.dram_tensor("ones", [1, 128], mm_dt, kind="ExternalInput").ap()
    # w1t columns are (m, g, o)-major so the m=0 block is one contiguous
    # 512-col DMA needed first; w2t columns are (g2, g, o)-major.
    w1t = nc.dram_tensor("w1t", [128, G * MB * 128], mm_dt, kind="ExternalInput").ap()
    w2t = nc.dram_tensor("w2t", [128, G * G * 128], mm_dt, kind="ExternalInput").ap()
    b1t = nc.dram_tensor("b1t", [128, G * MB], F32, kind="ExternalInput").ap()
    b2t = nc.dram_tensor("b2t", [128, G], F32, kind="ExternalInput").ap()
    outs = nc.dram_tensor("outs", [BPC * G, 128, T], mm_dt, kind="ExternalOutput").ap()

    with tile.TileContext(nc) as tc:
        with (
            tc.tile_pool(name="consts", bufs=1) as cpool,
            tc.tile_pool(name="xp", bufs=BPC * G) as xpool,
            tc.tile_pool(name="mbcp", bufs=2) as mbpool,
            tc.tile_pool(name="mkrp", bufs=2) as mkrpool,
            tc.tile_pool(name="hp", bufs=2 * G) as hpool,
            tc.tile_pool(name="op", bufs=2) as opool,
            tc.tile_pool(name="ps1p", bufs=3, space="PSUM") as ps1pool,
            tc.tile_pool(name="ps2p", bufs=2, space="PSUM") as ps2pool,
        ):
            # ones first (warm-up + mask broadcast depend on it)
            ones_sb = cpool.tile([1, 128], mm_dt)
            nc.sync.dma_start(ones_sb, ones)

            # PE warm-up: tiny matmuls on the ones row keep the HAM
            # activity window busy while real inputs stream in.
            wps = ps2pool.tile([128, 128], F32, tag="ps2", name="wps")
            for i in range(N_WARMUP):
                nc.tensor.matmul(
                    wps[:, 0:128], ones_sb, ones_sb, start=True, stop=True
                )

            w1_sb = cpool.tile([128, G * MB * 128], mm_dt)
            w2_sb = cpool.tile([128, G * G * 128], mm_dt)

            x_sb = [[None] * G for _ in range(BPC)]
            mask_bc = [None] * BPC

            def prep_batch(b):
                # mask row -> broadcast across 128 partitions via K=1
                # f16 matmuls; PSUM->SBUF copies must be on DVE (GpSimd
                # cannot access PSUM).
                mkrow = mkrpool.tile([1, T], mm_dt, tag="mkr", name="mkrow")
                nc.sync.dma_start(mkrow, mkr[b : b + 1, :])
                mbc = mbpool.tile([128, T], F32, tag="mbc", name="mbc")
                for c in range(NCH):
                    cs = slice(c * CH, (c + 1) * CH)
                    psb = ps2pool.tile([128, CH], F32, tag="ps2", name="psb")
                    nc.tensor.matmul(
                        psb, ones_sb, mkrow[:, cs], start=True, stop=True
                    )
                    nc.vector.tensor_copy(mbc[:, cs], psb)
                mask_bc[b] = mbc

            def load_x(b, g, split_first=False, ring=None):
                # ring: engine issuing the DMAs; half the tiles go out on
                # the GpSimd SWDGE ring so both rings issue in parallel
                ring = nc.sync if ring is None else ring
                xt = xpool.tile([128, T], mm_dt, tag="x", name="xt")
                if split_first:
                    # first tile in halves so GEMM1 can start sooner
                    for hh in range(2):
                        hs = slice(hh * (T // 2), (hh + 1) * (T // 2))
                        ring.dma_start(xt[:, hs], xs[b * G + g][:, hs])
                else:
                    ring.dma_start(xt, xs[b * G + g])
                x_sb[b][g] = xt

            def load_w1(m):
                ws = slice(m * G * 128, (m + 1) * G * 128)
                nc.sync.dma_start(w1_sb[:, ws], w1t[:, ws])

            def load_w2(g2, ring=None):
                ring = nc.sync if ring is None else ring
                ws = slice(g2 * G * 128, (g2 + 1) * G * 128)
                ring.dma_start(w2_sb[:, ws], w2t[:, ws])

            def gemm1_g(b, m, g):
                # one h tile (g) for (b, m), gelu+bias fused on drain
                ht = hpool.tile([128, T], mm_dt, tag="h", name="ht")
                w_ap = w1_sb[:, (m * G + g) * 128 : (m * G + g + 1) * 128]
                for half in range(T // GELU_W):
                    ps1 = ps1pool.tile([128, GELU_W], F32, tag="ps1", name="ps1")
                    for cc in range(GELU_W // CH):
                        c = half * (GELU_W // CH) + cc
                        nc.tensor.matmul(
                            ps1[:, cc * CH : (cc + 1) * CH],
                            w_ap,
                            x_sb[b][g][:, c * CH : (c + 1) * CH],
                            start=True, stop=True,
                        )
                    nc.scalar.activation(
                        ht[:, half * GELU_W : (half + 1) * GELU_W],
                        ps1,
                        mybir.ActivationFunctionType.Gelu,
                        bias=b1_sb[:, m * G + g : m * G + g + 1],
                        scale=1.0,
                    )
                return ht

            def gemm2_chunk(b, g2, hts, ot, c, och=1024):
                cs = slice(c * CH, (c + 1) * CH)
                ps2 = ps2pool.tile([128, CH], F32, tag="ps2", name="ps2")
                for g in range(G):
                    nc.tensor.matmul(
                        ps2,
                        w2_sb[:, (g2 * G + g) * 128 : (g2 * G + g + 1) * 128],
                        hts[g][:, cs],
                        start=(g == 0), stop=(g == G - 1),
                    )
                # out = (psum + b2) * mask, single fused DVE op
                nc.vector.scalar_tensor_tensor(
                    ot[:, cs],
                    ps2,
                    b2_sb[:, g2 : g2 + 1],
                    mask_bc[b][:, cs],
                    op0=mybir.AluOpType.add,
                    op1=mybir.AluOpType.mult,
                )
                if ((c + 1) * CH) % och == 0:
                    os_ = slice((c + 1) * CH - och, (c + 1) * CH)
                    nc.sync.dma_start(outs[b * G + g2][:, os_], ot[:, os_])

            # head: first w1 block + batch-0 x tiles split across both
            # rings so GEMM1 for (b=0, m=0) can start as early as
            # possible; everything else streams behind it.
            load_w1(0)
            load_x(0, 0, split_first=True)
            load_x(0, 2, split_first=True, ring=nc.gpsimd)
            load_x(0, 1)
            load_x(0, 3, ring=nc.gpsimd)
            b1_sb = cpool.tile([128, G * MB], F32)
            nc.sync.dma_start(b1_sb, b1t)
            b2_sb = cpool.tile([128, G], F32)
            nc.sync.dma_start(b2_sb, b2t)
            load_w1(1)
            load_w1(2)
            load_w1(3)
            prep_batch(0)
            for g2 in range(G):
                load_w2(g2, ring=nc.gpsimd if g2 >= 2 else None)

            # software pipeline over (b, m): GEMM2 chunks of iteration i-1
            # are interleaved between GEMM1 groups of iteration i so PE
            # alternates with ScalarE instead of stalling on gelu.
            prev = None
            for b in range(BPC):
                for m in range(MB):
                    hts = []
                    if prev is not None:
                        pot = opool.tile([128, T], mm_dt, tag="o", name="pot")
                    for g in range(G):
                        hts.append(gemm1_g(b, m, g))
                        if prev is not None:
                            gemm2_chunk(prev[0], prev[1], prev[2], pot, g)
                    prev = (b, m, hts)
                    if b + 1 < BPC and m == 1:
                        for g in range(G):
                            load_x(
                                b + 1, g,
                                ring=nc.gpsimd if g >= 2 else None,
                            )
                    if b + 1 < BPC and m == 2:
                        prep_batch(b + 1)
            pot = opool.tile([128, T], mm_dt, tag="o", name="pot")
            for c in range(NCH):
                gemm2_chunk(prev[0], prev[1], prev[2], pot, c, och=CH)

    nc.compile()
    return nc


def get_nc(mm_dt=None):
    mm_dt = MM_DT if mm_dt is None else mm_dt
    if mm_dt not in _compiled:
        _compiled[mm_dt] = _build(mm_dt)
    return _compiled[mm_dt]


def _np_dt(mm_dt):
    return np.float16 if mm_dt == F16 else np.float32


def prep_inputs(x, x_mask, w1, b1, w2, b2, mm_dt=None):
    """Host-side layout prep. Returns per-core in_maps."""
    mm_dt = MM_DT if mm_dt is None else mm_dt
    dt = _np_dt(mm_dt)
    x = np.ascontiguousarray(np.asarray(x, dtype=np.float32))
    x_mask = np.asarray(x_mask, dtype=np.float32)
    w1 = np.asarray(w1, dtype=np.float32)
    b1 = np.asarray(b1, dtype=np.float32)
    w2 = np.asarray(w2, dtype=np.float32)
    b2 = np.asarray(b2, dtype=np.float32)

    # w1 [H, CIN/G] -> lhsT blocks [i, (m, g, o)]
    w1r = w1.reshape(G, MB, 128, CIN // G)          # g, m, o, i
    w1t = np.ascontiguousarray(
        np.transpose(w1r, (3, 1, 0, 2)).reshape(128, G * MB * 128).astype(dt)
    )
    # w2 [COUT, H/G] -> lhsT blocks [i_local, (g2, g, o)]
    # GEMM2 group g2 contracts h tile (g, m=g2) row r against
    # w2[g2*128+o, r*4+g] (channel shuffle pre-applied).
    w2r = w2.reshape(G, 128, 128, G)                # g2, o, r, g
    w2t = np.ascontiguousarray(
        np.transpose(w2r, (2, 0, 3, 1)).reshape(128, G * G * 128).astype(dt)
    )
    b1tt = np.ascontiguousarray(
        b1.reshape(G, MB, 128).transpose(2, 1, 0).reshape(128, G * MB)
    )
    b2tt = np.ascontiguousarray(b2.reshape(G, 128).T)
    ones = np.ones((1, 128), dt)

    xr = x.astype(dt).reshape(N_CORES, BPC * G, 128, T)
    mr = x_mask.astype(dt).reshape(N_CORES, BPC, T)

    in_maps = []
    for k in range(N_CORES):
        in_maps.append(
            {
                "xs": np.ascontiguousarray(xr[k]),
                "mkr": np.ascontiguousarray(mr[k]),
                "ones": ones,
                "w1t": w1t,
                "w2t": w2t,
                "b1t": b1tt,
                "b2t": b2tt,
            }
        )
    return in_maps


def assemble_output(results):
    """results: list of 8 dicts with 'outs' [BPC*G, 128, T]."""
    parts = [
        r["outs"].astype(np.float32).reshape(BPC, G * 128, T) for r in results
    ]
    return np.concatenate(parts, axis=0)


def kernel(x, x_mask, w1, b1, w2, b2, n_groups):
    assert int(n_groups) == G
    import os

    # NTFF tracing needs antenv.axon_hooks, absent on this image; make
    # sure an inherited BASS_TRACE can't push us onto that path.
    os.environ["BASS_NEVER_TRACE"] = "1"
    nc = get_nc()
    in_maps = prep_inputs(x, x_mask, w1, b1, w2, b2)
    res = bass_utils.run_bass_kernel_spmd(
        nc, in_maps, core_ids=list(range(N_CORES))
    )
    return assemble_output(res.results)


# revision 6
# speedup vs baseline: 1.2303x; 1.0486x over previous
"""Trainium2 Bass kernel: grouped-pointwise FFN with channel shuffle.

Computes (per batch b, all ops pointwise in T):
    h   = W1_grouped @ x + b1                   # G=4 block-diagonal GEMM
    h   = channel_shuffle(h, G)
    h   = gelu(h)                               # exact erf gelu
    out = (W2_grouped @ h + b2) * mask

The reference computes mask*(f(mask*x)); for binary masks (the only
semantically valid values for a sequence mask) this equals mask*f(x),
so the input-side mask multiply is dropped and masking is applied only
on the output.

Sharding: data-parallel over batch B=16 across 8 cores (2 batches/core).
Weights are replicated; no collectives.

Layout on device (channel-partition):
  GEMM1: lhsT = w1 block [K=128(cin/G), M=128(out-ch block)],
         rhs  = x tile [128, 512(T chunk)], PSUM out [128, 512].
  gelu+bias fused on ScalarE reading PSUM [128, 1024] spans (2 banks).
  Channel shuffle is free: GEMM2's weight blocks are pre-gathered on the
  host so that GEMM2 group g2 contracts directly over GEMM1's (g, m=g2)
  output tiles.
  GEMM2: accumulate 4 K-blocks into PSUM [128, 512]; drain with a single
  fused DVE op: out = (psum + b2) * mask.

All matmul operands are float16 (1 cycle/row on PE, half the DMA bytes
and half the LDWEIGHTS time of fp32); PSUM accumulation is fp32.
Measured end-to-end rel err ~5e-4 vs the fp32 reference. Outputs are
stored fp16 and upcast on the host.

The software pipeline runs at half-tile (1024-column) granularity:
GEMM2 chunks of half-step i-1 interleave between GEMM1 halves of step
i, so the head fills and the tail drains in half an iteration. The x
tiles for the first batch stream as 1024-column halves, alternating
between the SP HWDGE ring (g=0,1) and the GpSimd SWDGE ring (g=2,3)
in the order GEMM1 consumes them. A dummy ACTIVATE right after the
first tiny DMA pulls the Gelu table load off the critical path, and a
burst of tiny warm-up matmuls keeps the PE p-state ramp warm while
the first inputs stream in.
"""

import numpy as np

import concourse.mybir as mybir
import concourse.tile as tile
from concourse import bacc
from concourse import bass_utils

F32 = mybir.dt.float32
F16 = mybir.dt.float16

N_CORES = 8
B, CIN, T = 16, 512, 2048
H, COUT, G = 2048, 512, 4
BPC = B // N_CORES        # batches per core
CH = 512                  # T chunk (= 1 PSUM bank of fp32)
NCH = T // CH             # 4 chunks
MB = (H // G) // 128      # 4 output-channel blocks per group in GEMM1
GELU_W = 1024             # ACT op width (2 PSUM banks)
N_WARMUP = 12             # tiny matmuls to warm the PE clock gate

MM_DT = F16

_compiled = {}


def _build(mm_dt):
    nc = bacc.Bacc(
        "TRN2", target_bir_lowering=False, debug=False, num_devices=N_CORES
    )
    xs = nc.dram_tensor("xs", [BPC * G, 128, T], mm_dt, kind="ExternalInput").ap()
    mkr = nc.dram_tensor("mkr", [BPC, T], mm_dt, kind="ExternalInput").ap()
    ones = nc.dram_tensor("ones", [1, 128], mm_dt, kind="ExternalInput").ap()
    # w1t columns are (m, g, o)-major so the m=0 block is one contiguous
    # 512-col DMA needed first; w2t columns are (g2, g, o)-major.
    w1t = nc.dram_tensor("w1t", [128, G * MB * 128], mm_dt, kind="ExternalInput").ap()
    w2t = nc.dram_tensor("w2t", [128, G * G * 128], mm_dt, kind="ExternalInput").ap()
    b1t = nc.dram_tensor("b1t", [128, G * MB], F32, kind="ExternalInput").ap()
    b2t = nc.dram_tensor("b2t", [128, G], F32, kind="ExternalInput").ap()
    outs = nc.dram_tensor("outs", [BPC * G, 128, T], mm_dt, kind="ExternalOutput").ap()

    with tile.TileContext(nc) as tc:
        with (
            tc.tile_pool(name="consts", bufs=1) as cpool,
            tc.tile_pool(name="xp", bufs=BPC * G) as xpool,
            tc.tile_pool(name="mbcp", bufs=2) as mbpool,
            tc.tile_pool(name="mkrp", bufs=2) as mkrpool,
            tc.tile_pool(name="hp", bufs=4 * G) as hpool,
            tc.tile_pool(name="op", bufs=2) as opool,
            tc.tile_pool(name="ps1p", bufs=3, space="PSUM") as ps1pool,
            tc.tile_pool(name="ps2p", bufs=2, space="PSUM") as ps2pool,
        ):
            # tiny DMAs first: ones (warm-up / broadcast lhsT) and the
            # batch-0 mask row
            ones_sb = cpool.tile([1, 128], mm_dt)
            nc.sync.dma_start(ones_sb, ones)
            mkrow0 = mkrpool.tile([1, T], mm_dt, tag="mkr", name="mkrow")
            nc.sync.dma_start(mkrow0, mkr[0:1, :])

            # dummy gelu on the ones row: loads the ACT Gelu table while
            # the real inputs still stream in
            scratch = cpool.tile([1, 128], mm_dt)
            nc.scalar.activation(
                scratch, ones_sb, mybir.ActivationFunctionType.Gelu
            )

            # PE warm-up: tiny matmuls on the ones row keep the HAM
            # activity window busy while real inputs stream in.
            wps = ps2pool.tile([128, 128], F32, tag="ps2", name="wps")
            for i in range(N_WARMUP):
                nc.tensor.matmul(
                    wps[:, 0:128], ones_sb, ones_sb, start=True, stop=True
                )

            w1_sb = cpool.tile([128, G * MB * 128], mm_dt)
            w2_sb = cpool.tile([128, G * G * 128], mm_dt)

            x_sb = [[None] * G for _ in range(BPC)]
            mask_bc = [None] * BPC

            def bcast_mask(b, mkrow):
                # mask row -> broadcast across 128 partitions via K=1
                # f16 matmuls; PSUM->SBUF copies must be on DVE (GpSimd
                # cannot access PSUM). Doubles as PE warm-up.
                mbc = mbpool.tile([128, T], F32, tag="mbc", name="mbc")
                for c in range(NCH):
                    cs = slice(c * CH, (c + 1) * CH)
                    psb = ps2pool.tile([128, CH], F32, tag="ps2", name="psb")
                    nc.tensor.matmul(
                        psb, ones_sb, mkrow[:, cs], start=True, stop=True
                    )
                    nc.vector.tensor_copy(mbc[:, cs], psb)
                mask_bc[b] = mbc

            def prep_batch(b):
                mkrow = mkrpool.tile([1, T], mm_dt, tag="mkr", name="mkrow")
                nc.sync.dma_start(mkrow, mkr[b : b + 1, :])
                bcast_mask(b, mkrow)

            def load_x(b, g, halves=False, ring=None):
                ring = nc.sync if ring is None else ring
                xt = xpool.tile([128, T], mm_dt, tag="x", name="xt")
                if halves:
                    for hh in range(2):
                        hs = slice(hh * (T // 2), (hh + 1) * (T // 2))
                        ring.dma_start(xt[:, hs], xs[b * G + g][:, hs])
                else:
                    ring.dma_start(xt, xs[b * G + g])
                x_sb[b][g] = xt

            def load_w1(m):
                ws = slice(m * G * 128, (m + 1) * G * 128)
                nc.sync.dma_start(w1_sb[:, ws], w1t[:, ws])

            def load_w2(g2, ring=None):
                ring = nc.sync if ring is None else ring
                ws = slice(g2 * G * 128, (g2 + 1) * G * 128)
                ring.dma_start(w2_sb[:, ws], w2t[:, ws])

            def gemm1_half(b, m, g, half):
                # one gelu half-tile [128, 1024] for (b, m, g, half)
                ht = hpool.tile([128, GELU_W], mm_dt, tag="h", name="ht")
                w_ap = w1_sb[:, (m * G + g) * 128 : (m * G + g + 1) * 128]
                ps1 = ps1pool.tile([128, GELU_W], F32, tag="ps1", name="ps1")
                for cc in range(GELU_W // CH):
                    c = half * (GELU_W // CH) + cc
                    nc.tensor.matmul(
                        ps1[:, cc * CH : (cc + 1) * CH],
                        w_ap,
                        x_sb[b][g][:, c * CH : (c + 1) * CH],
                        start=True, stop=True,
                    )
                nc.scalar.activation(
                    ht,
                    ps1,
                    mybir.ActivationFunctionType.Gelu,
                    bias=b1_sb[:, m * G + g : m * G + g + 1],
                    scale=1.0,
                )
                return ht

            def gemm2_chunk(b, g2, hhs, ot, c, och):
                # hhs: 4 gelu half-tiles [128, 1024] covering T columns
                # [ (c//2)*1024, (c//2+1)*1024 ); c is the global 512-col
                # chunk index in 0..3
                cs = slice(c * CH, (c + 1) * CH)
                hs = slice((c % 2) * CH, (c % 2 + 1) * CH)
                ps2 = ps2pool.tile([128, CH], F32, tag="ps2", name="ps2")
                for g in range(G):
                    nc.tensor.matmul(
                        ps2,
                        w2_sb[:, (g2 * G + g) * 128 : (g2 * G + g + 1) * 128],
                        hhs[g][:, hs],
                        start=(g == 0), stop=(g == G - 1),
                    )
                # out = (psum + b2) * mask, single fused DVE op
                nc.vector.scalar_tensor_tensor(
                    ot[:, cs],
                    ps2,
                    b2_sb[:, g2 : g2 + 1],
                    mask_bc[b][:, cs],
                    op0=mybir.AluOpType.add,
                    op1=mybir.AluOpType.mult,
                )
                if ((c + 1) * CH) % och == 0:
                    os_ = slice((c + 1) * CH - och, (c + 1) * CH)
                    nc.sync.dma_start(outs[b * G + g2][:, os_], ot[:, os_])

            # head DMA: w1 m=0 block + b1, then batch-0 x tiles as
            # 1024-col halves in GEMM1 consumption order (g0h0, g1h0,
            # g2h0, g3h0, g0h1, ...), g0/g1 on the SP ring and g2/g3 on
            # the GpSimd ring so both rings stream in parallel.
            load_w1(0)
            b1_sb = cpool.tile([128, G * MB], F32)
            nc.sync.dma_start(b1_sb, b1t)
            load_x(0, 0, halves=True)
            load_x(0, 2, halves=True, ring=nc.gpsimd)
            load_x(0, 1, halves=True)
            load_x(0, 3, halves=True, ring=nc.gpsimd)
            b2_sb = cpool.tile([128, G], F32)
            nc.sync.dma_start(b2_sb, b2t)
            load_w1(1)
            load_w1(2)
            load_w1(3)
            load_w2(0)
            load_w2(1)
            load_w2(2, ring=nc.gpsimd)
            load_w2(3, ring=nc.gpsimd)
            bcast_mask(0, mkrow0)

            # software pipeline at half-step granularity: GEMM2 chunks of
            # half-step i-1 interleave between GEMM1 halves of half-step
            # i so PE alternates with ScalarE instead of stalling on
            # gelu, and the tail only drains half an iteration.
            ots = {}

            def get_ot(b, m):
                if (b, m) not in ots:
                    ots[(b, m)] = opool.tile(
                        [128, T], mm_dt, tag="o", name="pot"
                    )
                return ots[(b, m)]

            prev = None
            for b in range(BPC):
                for m in range(MB):
                    hts = [[None] * 2 for _ in range(G)]
                    for half in range(2):
                        for g in range(G):
                            hts[g][half] = gemm1_half(b, m, g, half)
                            if prev is not None and g in (1, 3):
                                pb, pm, phts, phalf = prev
                                phhs = [phts[gg][phalf] for gg in range(G)]
                                c = phalf * 2 + (0 if g == 1 else 1)
                                gemm2_chunk(
                                    pb, pm, phhs, get_ot(pb, pm), c, och=1024
                                )
                        prev = (b, m, hts, half)
                    if b + 1 < BPC and m == 1:
                        for g in range(G):
                            load_x(
                                b + 1, g,
                                ring=nc.gpsimd if g >= 2 else None,
                            )
                    if b + 1 < BPC and m == 2:
                        prep_batch(b + 1)
            # final half-step's GEMM2, drained at 512-col granularity
            pb, pm, phts, phalf = prev
            phhs = [phts[gg][phalf] for gg in range(G)]
            for cc in range(2):
                gemm2_chunk(pb, pm, phhs, get_ot(pb, pm), phalf * 2 + cc, och=CH)

    nc.compile()
    return nc


def get_nc(mm_dt=None):
    mm_dt = MM_DT if mm_dt is None else mm_dt
    if mm_dt not in _compiled:
        _compiled[mm_dt] = _build(mm_dt)
    return _compiled[mm_dt]


def _np_dt(mm_dt):
    return np.float16 if mm_dt == F16 else np.float32


def prep_inputs(x, x_mask, w1, b1, w2, b2, mm_dt=None):
    """Host-side layout prep. Returns per-core in_maps."""
    mm_dt = MM_DT if mm_dt is None else mm_dt
    dt = _np_dt(mm_dt)
    x = np.ascontiguousarray(np.asarray(x, dtype=np.float32))
    x_mask = np.asarray(x_mask, dtype=np.float32)
    w1 = np.asarray(w1, dtype=np.float32)
    b1 = np.asarray(b1, dtype=np.float32)
    w2 = np.asarray(w2, dtype=np.float32)
    b2 = np.asarray(b2, dtype=np.float32)

    # w1 [H, CIN/G] -> lhsT blocks [i, (m, g, o)]
    w1r = w1.reshape(G, MB, 128, CIN // G)          # g, m, o, i
    w1t = np.ascontiguousarray(
        np.transpose(w1r, (3, 1, 0, 2)).reshape(128, G * MB * 128).astype(dt)
    )
    # w2 [COUT, H/G] -> lhsT blocks [i_local, (g2, g, o)]
    # GEMM2 group g2 contracts h tile (g, m=g2) row r against
    # w2[g2*128+o, r*4+g] (channel shuffle pre-applied).
    w2r = w2.reshape(G, 128, 128, G)                # g2, o, r, g
    w2t = np.ascontiguousarray(
        np.transpose(w2r, (2, 0, 3, 1)).reshape(128, G * G * 128).astype(dt)
    )
    b1tt = np.ascontiguousarray(
        b1.reshape(G, MB, 128).transpose(2, 1, 0).reshape(128, G * MB)
    )
    b2tt = np.ascontiguousarray(b2.reshape(G, 128).T)
    ones = np.ones((1, 128), dt)

    xr = x.astype(dt).reshape(N_CORES, BPC * G, 128, T)
    mr = x_mask.astype(dt).reshape(N_CORES, BPC, T)

    in_maps = []
    for k in range(N_CORES):
        in_maps.append(
            {
                "xs": np.ascontiguousarray(xr[k]),
                "mkr": np.ascontiguousarray(mr[k]),
                "ones": ones,
                "w1t": w1t,
                "w2t": w2t,
                "b1t": b1tt,
                "b2t": b2tt,
            }
        )
    return in_maps


def assemble_output(results):
    """results: list of 8 dicts with 'outs' [BPC*G, 128, T]."""
    parts = [
        r["outs"].astype(np.float32).reshape(BPC, G * 128, T) for r in results
    ]
    return np.concatenate(parts, axis=0)


def kernel(x, x_mask, w1, b1, w2, b2, n_groups):
    assert int(n_groups) == G
    import os

    # NTFF tracing needs antenv.axon_hooks, absent on this image; make
    # sure an inherited BASS_TRACE can't push us onto that path.
    os.environ["BASS_NEVER_TRACE"] = "1"
    nc = get_nc()
    in_maps = prep_inputs(x, x_mask, w1, b1, w2, b2)
    res = bass_utils.run_bass_kernel_spmd(
        nc, in_maps, core_ids=list(range(N_CORES))
    )
    return assemble_output(res.results)
